# revision 30
# baseline (speedup 1.0000x reference)
r"""Causal multi-head attention (B=4, T=2048, C=1024, H=16, D=64) on 8 TRN2 NeuronCores.

Sharding: core = (batch b, head-group hg).  b = core // 2, hg = core % 2.
Each core computes, for its batch, the attention-output contribution of its
8 heads, including the qkv projection restricted to those heads' columns and
the o-projection restricted to those heads' rows.  The two cores sharing a
batch produce partial sums of the o-projection; the host adds them together
with the (analytically folded) v-bias/o-bias correction.

Math notes:
  - k-bias contributes only q-row-constant score shifts, which cancel in
    softmax, so it is dropped; only the q bias is applied on device.
  - v bias and o bias are affine post-softmax:  (P@(V + 1 b_v^T))@Wo + b_o =
    (P@V)@Wo + (b_v@Wo + b_o), folded into a host-side correction row.
  - Softmax runs without max subtraction (scores are O(1)): e = exp(s/8).
    The denominator rides as PSUM rows 64-127 of the PV accumulation via 64
    all-ones columns appended to each V tile (lhsT M=128 costs nothing: the
    PE streams only the rhs free dim).

Engine schedule (the point of this version): the PE must never wait on the
softmax chain, and no side engine may become the bottleneck.
  - Lag-2 software pipeline in the attention inner loop: PV(tk-2) is issued
    after S(tk)/exp(tk), so every matmul's semaphores fire ~2 iterations
    early and consecutive matmuls overlap on the PE (fill/drain pipelining),
    like the projection phase achieves naturally.
  - PSUM: one shared pool of [128,1024] (2-bank) generations, bufs=3, used
    by S pairs, the qkv projection, and the o-projection; plus a
    single-buffered [128,1024] PV accumulator.  6 + 2 = 8 banks.
  - Both heads of a pair share one [128,1024] PSUM tile, so one exp
    instruction covers both S tiles (halves instruction overhead).
  - exp splits across two engines: ACT (table exp) takes most tiles; the
    DVE takes the triangular-masked diagonal blocks (a fused
    scalar_tensor_tensor computes a Schraudolph bit-trick exp:
    int16(s * 128*log2(e)/8 + bias) reinterpreted as bf16, where the bias
    tile holds 16249 on valid entries and -32000 on masked ones, which
    lands at -4e-36 ~ 0) plus overflow tiles when ACT would saturate.
  - Per head pair the PV PSUM is evacuated raw to SBUF staging (ACT copies
    one head, DVE the other) so the single-buffered PSUM frees fast; then
    GpSimd moves the denominator rows to partition base 0 (the custom-DVE
    reciprocal_approx_fast only works at base 0!), DVE takes the fast
    reciprocal, and a GpSimd multiply writes the normalized bf16 O^T.
  - Chunk 0 (all-diagonal, tiny PE window) is emitted in the middle of the
    V projection, hiding its side-engine load under the projection stream.
  - o-projection of chunk c-1 interleaves one token-tile per head-pair
    boundary inside chunk c; its PSUM is evacuated by the ACT engine.
  - Startup: only bq + x^T (one fused DMA) gate the first matmul; Wv / Wo
    loads are issued behind the first two weight-block loads.
"""

import sys

sys.path.insert(0, "/opt/trn_rl_repo")

import numpy as np

import concourse.bass as bass
import concourse.tile as tile
from concourse import bacc, mybir
from concourse.bass_utils import run_bass_kernel_spmd
from concourse.masks import make_upper_triangular

B, T, C = 4, 2048, 1024
H = 16
D = C // H          # 64
HL = 8              # heads per core
HD = HL * D         # 512: local head dim
N_CORES = 8
CB = C // 128       # 8 c-tiles
TQ_CH = T // 512    # 4 query chunks
TK_TILES = T // 128  # 16 key tiles

F32 = mybir.dt.float32
I16 = mybir.dt.int16
BF16 = mybir.dt.bfloat16

# Schraudolph exp-to-bf16 constants: bits(e^x) ~ int16(x*184.66496 + 16249)
# (128/ln 2 mantissa scale, 127*128 exponent bias, -7.3 sawtooth centering).
EXP_SCALE = 184.66496 * 0.125     # includes the 1/sqrt(D) score scaling
EXP_OFF = 16249.0
MASK_OFF = -32000.0               # bitcasts to -4e-36: an effective zero

_compiled = None
TRACE = False          # set True (e.g. from test.py) to neuron-profile the run
LAST_EXEC_NS = None    # filled with max per-core exec_time_ns when TRACE
LAST_TRACE = None      # (insts, trace_path) when TRACE


def _build():
    nc = bacc.Bacc("TRN2", target_bir_lowering=False, debug=False,
                   num_devices=N_CORES)

    xT_ap = nc.dram_tensor("xT", [C, T], BF16, kind="ExternalInput").ap()
    # wqk[n] = 128-column block n of [Wq_shard | Wk_shard], laid out
    # [n, cb, ci, j]: contraction c = cb*128 + ci, output column j.
    wqk_ap = nc.dram_tensor("wqk", [8, CB, 128, 128], BF16, kind="ExternalInput").ap()
    bq_ap = nc.dram_tensor("bq", [4, 128, 1], F32, kind="ExternalInput").ap()
    wv_ap = nc.dram_tensor("wv", [CB, 128, HD], BF16, kind="ExternalInput").ap()
    # wo[g] = rows of Wo for head pair g (head 2g rows 0-63, head 2g+1 rows 64-127)
    wo_ap = nc.dram_tensor("wo", [4, 128, C], BF16, kind="ExternalInput").ap()
    out_ap = nc.dram_tensor("out_p", [T, C], F32, kind="ExternalOutput").ap()

    with tile.TileContext(nc) as tc:
        with (
            tc.tile_pool(name="const", bufs=1) as const_pool,
            tc.tile_pool(name="qkt", bufs=1) as qkt_pool,
            tc.tile_pool(name="v", bufs=1) as v_pool,
            tc.tile_pool(name="ot", bufs=1) as ot_pool,
            tc.tile_pool(name="wow", bufs=1) as wo_pool,
            tc.tile_pool(name="e", bufs=4) as e_pool,
            tc.tile_pool(name="rec", bufs=2) as rec_pool,
            tc.tile_pool(name="stg", bufs=6) as stg_pool,
            tc.tile_pool(name="ostg", bufs=3) as ostg_pool,
            tc.tile_pool(name="ps", bufs=3, space="PSUM") as ps_pool,
            tc.tile_pool(name="ps_o", bufs=1, space="PSUM") as ps_o_pool,
        ):
            bqb = const_pool.tile([128, 4, 1], F32)
            nc.sync.dma_start(bqb[:], bq_ap.rearrange("n p o -> p n o"))
            # x^T in four DMAs on four different engine DGE queues: the
            # sync-queue ring serializes its transfers, so spreading across
            # engines is what actually parallelizes the load
            XT = const_pool.tile([128, CB, T], BF16, name="xt")
            for lo, hi, eng in ((0, 4, nc.sync), (4, 8, nc.scalar)):
                eng.dma_start(
                    XT[:, lo:hi, :],
                    xT_ap[lo * 128:hi * 128, :].rearrange(
                        "(cb p) t -> p cb t", p=128),
                )

            # Diagonal-block bias tile for the fused DVE exp: EXP_OFF where
            # key_i <= query_j, MASK_OFF elsewhere.
            offs = const_pool.tile([128, 128], F32)
            make_upper_triangular(nc, offs, val=EXP_OFF - MASK_OFF, diag=True)
            nc.vector.tensor_scalar_add(offs[:], offs[:], MASK_OFF)

            QKT = [qkt_pool.tile([128, T], BF16, name=f"qkt{n}") for n in range(8)]
            # V layout [128, 8 heads, 128]: cols 0-63 = head values, cols
            # 64-127 = 1.0 so the PV matmul emits 64 copies of the softmax
            # denominator as PSUM rows 64-127 of the same group.
            V = [v_pool.tile([128, HL, 2 * D], BF16, name=f"v{t}")
                 for t in range(TK_TILES)]
            for t in range(TK_TILES):
                nc.gpsimd.memset(V[t][:, :, D:2 * D], 1.0)
            OT = [ot_pool.tile([128, T], BF16, name=f"ot{g}") for g in range(4)]
            WV = wv_pool = const_pool.tile([128, CB, HD], BF16, name="wv")
            WO = const_pool.tile([128, 4, C], BF16, name="wo")

            # ACT-engine budget per chunk (ns): ~88% of the chunk's PE
            # window minus the ACT-side staging/o-proj evacuations.
            act_cap = [12000.0, 23000.0, 38500.0, 52500.0]
            act_used = [0.0]

            def exp_issue(q0, dst, src, width, diag_block):
                """Emit exp(src*0.125) -> dst (bf16) on ACT or DVE."""
                if diag_block:
                    nc.vector.scalar_tensor_tensor(
                        dst.bitcast(I16), src, EXP_SCALE, offs[:],
                        mybir.AluOpType.mult, mybir.AluOpType.add,
                    )
                    return
                cost_act = width * 0.833 + 180
                if act_used[0] + cost_act <= act_cap[q0]:
                    act_used[0] += cost_act
                    nc.scalar.activation(
                        dst, src, mybir.ActivationFunctionType.Exp,
                        scale=0.125,
                    )
                else:
                    nc.vector.tensor_scalar(
                        dst.bitcast(I16), src, EXP_SCALE, EXP_OFF,
                        mybir.AluOpType.mult, mybir.AluOpType.add,
                    )

            pending_fin = []

            def flush_fin(stage=None):
                while pending_fin and (stage is None or pending_fin[0][1] <= stage):
                    pending_fin.pop(0)[0]()

            def attention_chunk(q0, boundary, iter_hook=None):
                """S/exp/PV + normalize for one 512-query chunk.

                boundary(hp) is called after each head pair's normalize
                chain is issued (used to interleave prior-chunk o-proj);
                iter_hook() after every inner iteration (used to weave
                independent PE work through chunk 0's short pipeline).
                """
                tq0 = q0 * 512
                ntk = q0 * 4 + 4
                act_used[0] = 0.0
                for hp in range(HL // 2):        # head pairs: their K=64 S
                    ha, hb = 2 * hp, 2 * hp + 1  # matmuls use disjoint PE
                    qt = QKT[hp]                 # row groups -> concurrent
                    kt = QKT[4 + hp]
                    # both heads' PV accumulate into one 2-bank tile:
                    # cols 0-511 head a, cols 512-1023 head b
                    pso = ps_o_pool.tile([128, 2 * 512], F32, name="pso")

                    def issue_pv(item):
                        ptk, pj0, pe = item
                        nc.tensor.matmul(
                            pso[:, pj0:512], V[ptk][:, ha, :], pe[:, pj0:512],
                            start=(ptk == 0), stop=(ptk == ntk - 1),
                        )
                        nc.tensor.matmul(
                            pso[:, 512 + pj0:1024], V[ptk][:, hb, :],
                            pe[:, 512 + pj0:1024],
                            start=(ptk == 0), stop=(ptk == ntk - 1),
                        )

                    pend = []
                    for tk in range(ntk):
                        r = tk - q0 * 4
                        j0 = r * 128 if r >= 0 else 0
                        pss = ps_pool.tile([128, 2 * 512], F32, name="ps")
                        nc.tensor.matmul(
                            pss[:, j0:512],
                            kt[0:64, tk * 128:(tk + 1) * 128],
                            qt[0:64, tq0 + j0:tq0 + 512],
                            start=True, stop=True,
                        )
                        nc.tensor.matmul(
                            pss[:, 512 + j0:1024],
                            kt[64:128, tk * 128:(tk + 1) * 128],
                            qt[64:128, tq0 + j0:tq0 + 512],
                            start=True, stop=True,
                        )
                        e_t = e_pool.tile([128, 2 * 512], BF16, name="e_t")
                        if r >= 0:
                            for h0 in (0, 512):
                                exp_issue(q0, e_t[:, h0 + j0:h0 + j0 + 128],
                                          pss[:, h0 + j0:h0 + j0 + 128],
                                          128, True)
                                if j0 + 128 < 512:
                                    exp_issue(q0, e_t[:, h0 + j0 + 128:h0 + 512],
                                              pss[:, h0 + j0 + 128:h0 + 512],
                                              512 - j0 - 128, False)
                        else:
                            # one instruction covers both heads
                            exp_issue(q0, e_t[:], pss[:], 1024, False)
                        pend.append((tk, j0, e_t))
                        if len(pend) > 2:
                            issue_pv(pend.pop(0))
                        if tk == 1:
                            # the previous head pair's finalize is staged
                            # into this pair's stream: evacuations (freeing
                            # the PV PSUM) go out now, after this pair's
                            # first exps are in the DVE queue...
                            flush_fin(1)
                        elif tk == 3:
                            # ...and the reciprocals two iterations later,
                            # when their denominator inputs are long ready,
                            # so they retire instantly instead of
                            # head-of-line-blocking queued DVE exps
                            flush_fin(2)
                        if iter_hook is not None:
                            iter_hook()
                    for item in pend:
                        issue_pv(item)
                    tail = q0 == TQ_CH - 1
                    stgs = []

                    def fin1(pso=pso, tail=tail):
                        # evacuate raw PV+denominator to SBUF staging (frees
                        # the single-buffered PSUM; ACT head a, DVE head b),
                        # and start the GpSimd denominator moves down to
                        # partition base 0 (the custom-DVE
                        # reciprocal_approx_fast only works at base 0)
                        stg_a = stg_pool.tile([128, 512], F32, name="stg_a")
                        stg_b = stg_pool.tile([128, 512], F32, name="stg_b")
                        nc.scalar.copy(stg_a[:], pso[:, 0:512])
                        nc.vector.tensor_copy(stg_b[:], pso[:, 512:1024])
                        for po, stg in ((0, stg_a), (64, stg_b)):
                            den0 = rec_pool.tile([64, 512], F32, name=f"den{po}")
                            (nc.vector if tail else nc.gpsimd).tensor_copy(
                                den0[:], stg[64:128, :])
                            stgs.append((po, stg, den0))

                    def fin2(hp=hp, tail=tail):
                        for po, stg, den0 in stgs:
                            rec = rec_pool.tile([64, 512], F32, name=f"rec{po}")
                            nc.vector.reciprocal_approx_fast(rec[:], den0[:])
                            (nc.vector if tail else nc.gpsimd).tensor_mul(
                                OT[hp][po:po + 64, tq0:tq0 + 512],
                                stg[0:64, :], rec[:],
                            )
                        boundary(hp)
                    if q0 == TQ_CH - 1 and hp == HL // 2 - 1:
                        fin1()
                        fin2()
                    else:
                        pending_fin.append((fin1, 1))
                        pending_fin.append((fin2, 2))

            def oproj(tt, evac=None):
                psp = ps_pool.tile([128, 2 * 512], F32, name="ps")
                for half in range(2):
                    n0 = half * 512
                    for g in range(4):
                        nc.tensor.matmul(
                            psp[:, n0:n0 + 512],
                            OT[g][:, tt * 128:(tt + 1) * 128],
                            WO[:, g, n0:n0 + 512],
                            start=(g == 0), stop=(g == 3),
                        )
                ob = ostg_pool.tile([128, 2 * 512], F32, name="ob")
                if evac is None:
                    nc.scalar.copy(ob[:], psp[:])
                    act_used[0] += 1000
                else:
                    evac(ob[:], psp[:])
                nc.sync.dma_start(out_ap[tt * 128:(tt + 1) * 128, :], ob[:])

            # ---------------- projection phase (+ chunk 0) ----------------
            with (
                tc.tile_pool(name="wqk", bufs=2) as wqk_pool,
            ):
                # QK^T = W^T @ x^T, output rows = qk columns (col block n)
                for n in range(8):
                    wt = wqk_pool.tile([128, CB, 128], BF16)
                    nc.sync.dma_start(wt[:], wqk_ap[n].rearrange("cb ci j -> ci cb j"))
                    if n == 1:
                        # queue the big weight loads behind the first blocks
                        nc.sync.dma_start(
                            WV[:], wv_ap.rearrange("cb p h -> p cb h"))
                    elif n == 2:
                        nc.sync.dma_start(
                            WO[:], wo_ap.rearrange("g p c -> p g c"))
                    for q0 in range(TQ_CH):
                        ps = ps_pool.tile([128, 2 * 512], F32, name="ps")
                        for cb in range(CB):
                            nc.tensor.matmul(
                                ps[:, 0:512], wt[:, cb, :],
                                XT[:, cb, q0 * 512:(q0 + 1) * 512],
                                start=(cb == 0), stop=(cb == CB - 1),
                            )
                        dst = QKT[n][:, q0 * 512:(q0 + 1) * 512]
                        if n < 4:
                            nc.vector.tensor_scalar_add(
                                dst, ps[:, 0:512], bqb[:, n, :])
                        else:
                            # plain cast: use the otherwise-idle ACT engine
                            nc.scalar.copy(dst, ps[:, 0:512])

                # V = x @ Wv (natural layout)
                def vproj(tt):
                    ps = ps_pool.tile([128, 2 * 512], F32, name="ps")
                    for cb in range(CB):
                        nc.tensor.matmul(
                            ps[:, 0:512], XT[:, cb, tt * 128:(tt + 1) * 128],
                            WV[:, cb, :],
                            start=(cb == 0), stop=(cb == CB - 1),
                        )
                    # tiles woven through chunk 0 evacuate via ACT: the DVE
                    # is loaded with chunk 0's fused mask-exps there
                    (nc.scalar.copy if tt >= 4 else nc.vector.tensor_copy)(
                        V[tt][:, :, 0:D],
                        ps[:, 0:512].rearrange("p (h d) -> p h d", h=HL),
                    )

                for tt in range(4):
                    vproj(tt)
                # chunk 0 runs with the remaining V projection woven through
                # its iterations: the dependency-free projection groups keep
                # the PE streaming across chunk 0's short softmax pipeline
                pending_v = list(range(4, TK_TILES))
                it_count = [0]

                def weave():
                    it_count[0] += 1
                    if pending_v and it_count[0] % 4 != 0:
                        vproj(pending_v.pop(0))

                attention_chunk(0, lambda hp: None, weave)
                for tt in pending_v:
                    vproj(tt)

            # ---------------- attention + output projection ----------------
            for q0 in range(1, TQ_CH):
                attention_chunk(q0, lambda hp, c=q0: oproj(4 * (c - 1) + hp))
            flush_fin()
            for tt in range(12, 16):
                oproj(tt, evac=nc.vector.tensor_copy)

    nc.compile()
    return nc


def _prep_core_inputs(hidden_state, qkv_w, qkv_b, o_w, b, hg):
    """Build the per-core input map for batch b, head group hg."""
    import ml_dtypes
    bf16 = ml_dtypes.bfloat16
    s = slice(hg * HD, (hg + 1) * HD)
    wq = qkv_w[:, 0 * C:1 * C][:, s]          # [C, 512]
    wk = qkv_w[:, 1 * C:2 * C][:, s]          # [C, 512]
    wv = qkv_w[:, 2 * C:3 * C][:, s]          # [C, 512]
    bq = qkv_b[0 * C:1 * C][s]                # [512]

    wqk = np.concatenate([wq, wk], axis=1)    # [C, 1024]
    # [n, cb, ci, j]
    wqk_r = np.ascontiguousarray(
        wqk.reshape(CB, 128, 8, 128).transpose(2, 0, 1, 3)
    )
    bq_r = np.ascontiguousarray(bq.reshape(4, 128, 1))
    wv_r = np.ascontiguousarray(wv.reshape(CB, 128, HD))
    # o_w rows for this head group, regrouped [g, 128, C] in head-pair order
    wo = o_w[hg * HD:(hg + 1) * HD, :]        # [512, C]
    wo_r = np.ascontiguousarray(wo.reshape(4, 128, C))

    xT = np.ascontiguousarray(hidden_state[b].T)  # [C, T]
    return {
        "xT": xT.astype(bf16),
        "wqk": wqk_r.astype(bf16),
        "bq": bq_r.astype(np.float32),
        "wv": wv_r.astype(bf16),
        "wo": wo_r.astype(bf16),
    }


def _ensure_profile_hook():
    """Register the NTFF profiling hook that this container's antenv lacks.

    The axon boot code registers it via ``antenv.axon_hooks`` when that
    module exists; here we synthesize the module and point it at the same
    ctypes shim over libaxon_pjrt.so.
    """
    import types
    try:
        from antenv.axon_hooks import get_axon_ntff_profile_hook  # noqa: F401
        return
    except ImportError:
        pass
    try:
        import antenv
        from trn_agent_boot.trn_boot import _ntff_profile_via_ctypes
        hook = {"h": _ntff_profile_via_ctypes("/opt/axon/libaxon_pjrt.so")}
        mod = types.ModuleType("antenv.axon_hooks")
        mod.set_axon_ntff_profile_hook = lambda h: hook.__setitem__("h", h)
        mod.get_axon_ntff_profile_hook = lambda: hook["h"]
        sys.modules["antenv.axon_hooks"] = mod
        antenv.axon_hooks = mod
    except Exception as e:  # profiling is best-effort
        print(f"profile hook setup failed: {e}", flush=True)


def kernel(hidden_state, qkv_w, qkv_b, o_w, o_b):
    global _compiled
    hidden_state = np.asarray(hidden_state, dtype=np.float32)
    qkv_w = np.asarray(qkv_w, dtype=np.float32)
    qkv_b = np.asarray(qkv_b, dtype=np.float32)
    o_w = np.asarray(o_w, dtype=np.float32)
    o_b = np.asarray(o_b, dtype=np.float32)

    if _compiled is None:
        _compiled = _build()
    nc = _compiled

    in_maps = []
    for core in range(N_CORES):
        b, hg = core // 2, core % 2
        in_maps.append(_prep_core_inputs(hidden_state, qkv_w, qkv_b, o_w, b, hg))

    global LAST_EXEC_NS, LAST_TRACE
    kw = {}
    if TRACE:
        import tempfile
        _ensure_profile_hook()
        kw = dict(trace=True, tmpdir=tempfile.mkdtemp(prefix="bass_attn_trace_"))
    res = run_bass_kernel_spmd(nc, in_maps, core_ids=list(range(N_CORES)), **kw)
    LAST_EXEC_NS = res.exec_time_ns
    LAST_TRACE = res.instructions_and_trace

    # host-side gather: sum the two head-group partials per batch and add the
    # affine correction (v-bias pushed through Wo, plus o-bias).
    bv = qkv_b[2 * C:3 * C]                   # [C]
    corr = (bv @ o_w + o_b).astype(np.float32)
    out = np.empty((B, T, C), dtype=np.float32)
    for b in range(B):
        p0 = res.results[2 * b]["out_p"]
        p1 = res.results[2 * b + 1]["out_p"]
        out[b] = p0 + p1 + corr
    return out


# revision 31
# speedup vs baseline: 1.0102x; 1.0102x over previous
r"""Causal multi-head attention (B=4, T=2048, C=1024, H=16, D=64) on 8 TRN2 NeuronCores.

Sharding: core = (batch b, head-group hg).  b = core // 2, hg = core % 2.
Each core computes, for its batch, the attention-output contribution of its
8 heads, including the qkv projection restricted to those heads' columns and
the o-projection restricted to those heads' rows.  The two cores sharing a
batch produce partial sums of the o-projection; the host adds them together
with the (analytically folded) v-bias/o-bias correction.

Math notes:
  - k-bias contributes only q-row-constant score shifts, which cancel in
    softmax, so it is dropped; only the q bias is applied on device.
  - v bias and o bias are affine post-softmax:  (P@(V + 1 b_v^T))@Wo + b_o =
    (P@V)@Wo + (b_v@Wo + b_o), folded into a host-side correction row.
  - Softmax runs without max subtraction (scores are O(1)): e = exp(s/8).
    The denominator rides as PSUM rows 64-127 of the PV accumulation via 64
    all-ones columns appended to each V tile (lhsT M=128 costs nothing: the
    PE streams only the rhs free dim).

Engine schedule (the point of this version): the PE must never wait on the
softmax chain, and no side engine may become the bottleneck.
  - Lag-2 software pipeline in the attention inner loop: PV(tk-2) is issued
    after S(tk)/exp(tk), so every matmul's semaphores fire ~2 iterations
    early and consecutive matmuls overlap on the PE (fill/drain pipelining),
    like the projection phase achieves naturally.
  - PSUM: one shared pool of [128,1024] (2-bank) generations, bufs=3, used
    by S pairs, the qkv projection, and the o-projection; plus a
    single-buffered [128,1024] PV accumulator.  6 + 2 = 8 banks.
  - Both heads of a pair share one [128,1024] PSUM tile, so one exp
    instruction covers both S tiles (halves instruction overhead).
  - exp splits across two engines: ACT (table exp) takes most tiles; the
    DVE takes the triangular-masked diagonal blocks (a fused
    scalar_tensor_tensor computes a Schraudolph bit-trick exp:
    int16(s * 128*log2(e)/8 + bias) reinterpreted as bf16, where the bias
    tile holds 16249 on valid entries and -32000 on masked ones, which
    lands at -4e-36 ~ 0) plus overflow tiles when ACT would saturate.
  - Per head pair the PV PSUM is evacuated raw to SBUF staging (ACT copies
    one head, DVE the other) so the single-buffered PSUM frees fast; then
    GpSimd moves the denominator rows to partition base 0 (the custom-DVE
    reciprocal_approx_fast only works at base 0!), DVE takes the fast
    reciprocal, and a GpSimd multiply writes the normalized bf16 O^T.
  - Chunk 0 (all-diagonal, tiny PE window) is emitted in the middle of the
    V projection, hiding its side-engine load under the projection stream.
  - o-projection of chunk c-1 interleaves one token-tile per head-pair
    boundary inside chunk c; its PSUM is evacuated by the ACT engine.
  - Startup: only bq + x^T (one fused DMA) gate the first matmul; Wv / Wo
    loads are issued behind the first two weight-block loads.
"""

import sys

sys.path.insert(0, "/opt/trn_rl_repo")

import numpy as np

import concourse.bass as bass
import concourse.tile as tile
from concourse import bacc, mybir
from concourse.bass_utils import run_bass_kernel_spmd
from concourse.masks import make_upper_triangular

B, T, C = 4, 2048, 1024
H = 16
D = C // H          # 64
HL = 8              # heads per core
HD = HL * D         # 512: local head dim
N_CORES = 8
CB = C // 128       # 8 c-tiles
TQ_CH = T // 512    # 4 query chunks
TK_TILES = T // 128  # 16 key tiles

F32 = mybir.dt.float32
I16 = mybir.dt.int16
BF16 = mybir.dt.bfloat16

# Schraudolph exp-to-bf16 constants: bits(e^x) ~ int16(x*184.66496 + 16249)
# (128/ln 2 mantissa scale, 127*128 exponent bias, -7.3 sawtooth centering).
EXP_SCALE = 184.66496 * 0.125     # includes the 1/sqrt(D) score scaling
EXP_OFF = 16249.0
MASK_OFF = -32000.0               # bitcasts to -4e-36: an effective zero

_compiled = None
TRACE = False          # set True (e.g. from test.py) to neuron-profile the run
LAST_EXEC_NS = None    # filled with max per-core exec_time_ns when TRACE
LAST_TRACE = None      # (insts, trace_path) when TRACE


def _build():
    nc = bacc.Bacc("TRN2", target_bir_lowering=False, debug=False,
                   num_devices=N_CORES)

    xT_ap = nc.dram_tensor("xT", [C, T], BF16, kind="ExternalInput").ap()
    # wqk[n] = 128-column block n of [Wq_shard | Wk_shard], laid out
    # [n, cb, ci, j]: contraction c = cb*128 + ci, output column j.
    wqk_ap = nc.dram_tensor("wqk", [8, CB, 128, 128], BF16, kind="ExternalInput").ap()
    bq_ap = nc.dram_tensor("bq", [4, 128, 1], F32, kind="ExternalInput").ap()
    wv_ap = nc.dram_tensor("wv", [CB, 128, HD], BF16, kind="ExternalInput").ap()
    # wo[g] = rows of Wo for head pair g (head 2g rows 0-63, head 2g+1 rows 64-127)
    wo_ap = nc.dram_tensor("wo", [4, 128, C], BF16, kind="ExternalInput").ap()
    out_ap = nc.dram_tensor("out_p", [T, C], F32, kind="ExternalOutput").ap()

    with tile.TileContext(nc) as tc:
        with (
            tc.tile_pool(name="const", bufs=1) as const_pool,
            tc.tile_pool(name="qkt", bufs=1) as qkt_pool,
            tc.tile_pool(name="v", bufs=1) as v_pool,
            tc.tile_pool(name="ot", bufs=1) as ot_pool,
            tc.tile_pool(name="wow", bufs=1) as wo_pool,
            tc.tile_pool(name="e", bufs=4) as e_pool,
            tc.tile_pool(name="rec", bufs=2) as rec_pool,
            tc.tile_pool(name="stg", bufs=6) as stg_pool,
            tc.tile_pool(name="ostg", bufs=3) as ostg_pool,
            tc.tile_pool(name="ps", bufs=3, space="PSUM") as ps_pool,
            tc.tile_pool(name="ps_o", bufs=1, space="PSUM") as ps_o_pool,
        ):
            bqb = const_pool.tile([128, 4, 1], F32)
            nc.sync.dma_start(bqb[:], bq_ap.rearrange("n p o -> p n o"))
            # x^T in four DMAs on four different engine DGE queues: the
            # sync-queue ring serializes its transfers, so spreading across
            # engines is what actually parallelizes the load
            XT = const_pool.tile([128, CB, T], BF16, name="xt")
            for lo, hi, eng in ((0, 4, nc.sync), (4, 8, nc.scalar)):
                eng.dma_start(
                    XT[:, lo:hi, :],
                    xT_ap[lo * 128:hi * 128, :].rearrange(
                        "(cb p) t -> p cb t", p=128),
                )

            # Diagonal-block bias tile for the fused DVE exp: EXP_OFF where
            # key_i <= query_j, MASK_OFF elsewhere.
            offs = const_pool.tile([128, 128], F32)
            make_upper_triangular(nc, offs, val=EXP_OFF - MASK_OFF, diag=True)
            nc.vector.tensor_scalar_add(offs[:], offs[:], MASK_OFF)

            QKT = [qkt_pool.tile([128, T], BF16, name=f"qkt{n}") for n in range(8)]
            # V layout [128, 8 heads, 128]: cols 0-63 = head values, cols
            # 64-127 = 1.0 so the PV matmul emits 64 copies of the softmax
            # denominator as PSUM rows 64-127 of the same group.
            V = [v_pool.tile([128, HL, 2 * D], BF16, name=f"v{t}")
                 for t in range(TK_TILES)]
            for t in range(TK_TILES):
                nc.gpsimd.memset(V[t][:, :, D:2 * D], 1.0)
            OT = [ot_pool.tile([128, T], BF16, name=f"ot{g}") for g in range(4)]
            WV = wv_pool = const_pool.tile([128, CB, HD], BF16, name="wv")
            WO = const_pool.tile([128, 4, C], BF16, name="wo")

            # ACT-engine budget per chunk (ns): ~88% of the chunk's PE
            # window minus the ACT-side staging/o-proj evacuations.
            act_cap = [12000.0, 24500.0, 41000.0, 56000.0]
            act_used = [0.0]

            def exp_issue(q0, dst, src, width, diag_block):
                """Emit exp(src*0.125) -> dst (bf16) on ACT or DVE."""
                if diag_block:
                    nc.vector.scalar_tensor_tensor(
                        dst.bitcast(I16), src, EXP_SCALE, offs[:],
                        mybir.AluOpType.mult, mybir.AluOpType.add,
                    )
                    return
                cost_act = width * 0.833 + 180
                if act_used[0] + cost_act <= act_cap[q0]:
                    act_used[0] += cost_act
                    nc.scalar.activation(
                        dst, src, mybir.ActivationFunctionType.Exp,
                        scale=0.125,
                    )
                else:
                    nc.vector.tensor_scalar(
                        dst.bitcast(I16), src, EXP_SCALE, EXP_OFF,
                        mybir.AluOpType.mult, mybir.AluOpType.add,
                    )

            pending_fin = []

            def flush_fin(stage=None):
                while pending_fin and (stage is None or pending_fin[0][1] <= stage):
                    pending_fin.pop(0)[0]()

            def attention_chunk(q0, boundary, iter_hook=None):
                """S/exp/PV + normalize for one 512-query chunk.

                boundary(hp) is called after each head pair's normalize
                chain is issued (used to interleave prior-chunk o-proj);
                iter_hook() after every inner iteration (used to weave
                independent PE work through chunk 0's short pipeline).
                """
                tq0 = q0 * 512
                ntk = q0 * 4 + 4
                act_used[0] = 0.0
                for hp in range(HL // 2):        # head pairs: their K=64 S
                    ha, hb = 2 * hp, 2 * hp + 1  # matmuls use disjoint PE
                    qt = QKT[hp]                 # row groups -> concurrent
                    kt = QKT[4 + hp]
                    # both heads' PV accumulate into one 2-bank tile:
                    # cols 0-511 head a, cols 512-1023 head b
                    pso = ps_o_pool.tile([128, 2 * 512], F32, name="pso")

                    def issue_pv(item):
                        ptk, pj0, pe = item
                        nc.tensor.matmul(
                            pso[:, pj0:512], V[ptk][:, ha, :], pe[:, pj0:512],
                            start=(ptk == 0), stop=(ptk == ntk - 1),
                        )
                        nc.tensor.matmul(
                            pso[:, 512 + pj0:1024], V[ptk][:, hb, :],
                            pe[:, 512 + pj0:1024],
                            start=(ptk == 0), stop=(ptk == ntk - 1),
                        )

                    pend = []
                    for tk in range(ntk):
                        r = tk - q0 * 4
                        j0 = r * 128 if r >= 0 else 0
                        pss = ps_pool.tile([128, 2 * 512], F32, name="ps")
                        nc.tensor.matmul(
                            pss[:, j0:512],
                            kt[0:64, tk * 128:(tk + 1) * 128],
                            qt[0:64, tq0 + j0:tq0 + 512],
                            start=True, stop=True,
                        )
                        nc.tensor.matmul(
                            pss[:, 512 + j0:1024],
                            kt[64:128, tk * 128:(tk + 1) * 128],
                            qt[64:128, tq0 + j0:tq0 + 512],
                            start=True, stop=True,
                        )
                        e_t = e_pool.tile([128, 2 * 512], BF16, name="e_t")
                        if r >= 0:
                            for h0 in (0, 512):
                                exp_issue(q0, e_t[:, h0 + j0:h0 + j0 + 128],
                                          pss[:, h0 + j0:h0 + j0 + 128],
                                          128, True)
                                if j0 + 128 < 512:
                                    exp_issue(q0, e_t[:, h0 + j0 + 128:h0 + 512],
                                              pss[:, h0 + j0 + 128:h0 + 512],
                                              512 - j0 - 128, False)
                        else:
                            # one instruction covers both heads
                            exp_issue(q0, e_t[:], pss[:], 1024, False)
                        pend.append((tk, j0, e_t))
                        if len(pend) > 2:
                            issue_pv(pend.pop(0))
                        if tk == 1:
                            # the previous head pair's finalize is staged
                            # into this pair's stream: evacuations (freeing
                            # the PV PSUM) go out now, after this pair's
                            # first exps are in the DVE queue...
                            flush_fin(1)
                        elif tk == 3:
                            # ...and the reciprocals two iterations later,
                            # when their denominator inputs are long ready,
                            # so they retire instantly instead of
                            # head-of-line-blocking queued DVE exps
                            flush_fin(2)
                        if iter_hook is not None:
                            iter_hook()
                    for item in pend:
                        issue_pv(item)
                    tail = q0 == TQ_CH - 1
                    stgs = []

                    def fin1(pso=pso, tail=tail):
                        # evacuate raw PV+denominator to SBUF staging (frees
                        # the single-buffered PSUM; ACT head a, DVE head b),
                        # and start the GpSimd denominator moves down to
                        # partition base 0 (the custom-DVE
                        # reciprocal_approx_fast only works at base 0)
                        stg_a = stg_pool.tile([128, 512], F32, name="stg_a")
                        stg_b = stg_pool.tile([128, 512], F32, name="stg_b")
                        nc.scalar.copy(stg_a[:], pso[:, 0:512])
                        nc.vector.tensor_copy(stg_b[:], pso[:, 512:1024])
                        for po, stg in ((0, stg_a), (64, stg_b)):
                            den0 = rec_pool.tile([64, 512], F32, name=f"den{po}")
                            (nc.vector if tail else nc.gpsimd).tensor_copy(
                                den0[:], stg[64:128, :])
                            stgs.append((po, stg, den0))

                    def fin2(hp=hp, tail=tail):
                        for po, stg, den0 in stgs:
                            rec = rec_pool.tile([64, 512], F32, name=f"rec{po}")
                            nc.vector.reciprocal_approx_fast(rec[:], den0[:])
                            (nc.vector if tail else nc.gpsimd).tensor_mul(
                                OT[hp][po:po + 64, tq0:tq0 + 512],
                                stg[0:64, :], rec[:],
                            )
                        boundary(hp)
                    if q0 == TQ_CH - 1 and hp == HL // 2 - 1:
                        fin1()
                        fin2()
                    else:
                        pending_fin.append((fin1, 1))
                        pending_fin.append((fin2, 2))

            def oproj(tt, evac=None):
                psp = ps_pool.tile([128, 2 * 512], F32, name="ps")
                for half in range(2):
                    n0 = half * 512
                    for g in range(4):
                        nc.tensor.matmul(
                            psp[:, n0:n0 + 512],
                            OT[g][:, tt * 128:(tt + 1) * 128],
                            WO[:, g, n0:n0 + 512],
                            start=(g == 0), stop=(g == 3),
                        )
                ob = ostg_pool.tile([128, 2 * 512], F32, name="ob")
                if evac is None:
                    nc.scalar.copy(ob[:], psp[:])
                    act_used[0] += 1000
                else:
                    evac(ob[:], psp[:])
                nc.sync.dma_start(out_ap[tt * 128:(tt + 1) * 128, :], ob[:])

            # ---------------- projection phase (+ chunk 0) ----------------
            with (
                tc.tile_pool(name="wqk", bufs=2) as wqk_pool,
            ):
                # QK^T = W^T @ x^T, output rows = qk columns (col block n)
                for n in range(8):
                    wt = wqk_pool.tile([128, CB, 128], BF16)
                    nc.sync.dma_start(wt[:], wqk_ap[n].rearrange("cb ci j -> ci cb j"))
                    if n == 1:
                        # queue the big weight loads behind the first blocks
                        nc.sync.dma_start(
                            WV[:], wv_ap.rearrange("cb p h -> p cb h"))
                    elif n == 2:
                        nc.sync.dma_start(
                            WO[:], wo_ap.rearrange("g p c -> p g c"))
                    for q0 in range(TQ_CH):
                        ps = ps_pool.tile([128, 2 * 512], F32, name="ps")
                        for cb in range(CB):
                            nc.tensor.matmul(
                                ps[:, 0:512], wt[:, cb, :],
                                XT[:, cb, q0 * 512:(q0 + 1) * 512],
                                start=(cb == 0), stop=(cb == CB - 1),
                            )
                        dst = QKT[n][:, q0 * 512:(q0 + 1) * 512]
                        if n < 4:
                            nc.vector.tensor_scalar_add(
                                dst, ps[:, 0:512], bqb[:, n, :])
                        else:
                            # plain cast: use the otherwise-idle ACT engine
                            nc.scalar.copy(dst, ps[:, 0:512])

                # V = x @ Wv (natural layout)
                def vproj(tt):
                    ps = ps_pool.tile([128, 2 * 512], F32, name="ps")
                    for cb in range(CB):
                        nc.tensor.matmul(
                            ps[:, 0:512], XT[:, cb, tt * 128:(tt + 1) * 128],
                            WV[:, cb, :],
                            start=(cb == 0), stop=(cb == CB - 1),
                        )
                    # tiles woven through chunk 0 evacuate via ACT: the DVE
                    # is loaded with chunk 0's fused mask-exps there
                    (nc.scalar.copy if tt >= 4 else nc.vector.tensor_copy)(
                        V[tt][:, :, 0:D],
                        ps[:, 0:512].rearrange("p (h d) -> p h d", h=HL),
                    )

                for tt in range(4):
                    vproj(tt)
                # chunk 0 runs with the remaining V projection woven through
                # its iterations: the dependency-free projection groups keep
                # the PE streaming across chunk 0's short softmax pipeline
                pending_v = list(range(4, TK_TILES))
                it_count = [0]

                def weave():
                    it_count[0] += 1
                    if pending_v and it_count[0] % 4 != 0:
                        vproj(pending_v.pop(0))

                attention_chunk(0, lambda hp: None, weave)
                for tt in pending_v:
                    vproj(tt)

            # ---------------- attention + output projection ----------------
            for q0 in range(1, TQ_CH):
                attention_chunk(q0, lambda hp, c=q0: oproj(4 * (c - 1) + hp))
            flush_fin()
            for tt in range(12, 16):
                oproj(tt, evac=nc.vector.tensor_copy)

    nc.compile()
    return nc


def _prep_core_inputs(hidden_state, qkv_w, qkv_b, o_w, b, hg):
    """Build the per-core input map for batch b, head group hg."""
    import ml_dtypes
    bf16 = ml_dtypes.bfloat16
    s = slice(hg * HD, (hg + 1) * HD)
    wq = qkv_w[:, 0 * C:1 * C][:, s]          # [C, 512]
    wk = qkv_w[:, 1 * C:2 * C][:, s]          # [C, 512]
    wv = qkv_w[:, 2 * C:3 * C][:, s]          # [C, 512]
    bq = qkv_b[0 * C:1 * C][s]                # [512]

    wqk = np.concatenate([wq, wk], axis=1)    # [C, 1024]
    # [n, cb, ci, j]
    wqk_r = np.ascontiguousarray(
        wqk.reshape(CB, 128, 8, 128).transpose(2, 0, 1, 3)
    )
    bq_r = np.ascontiguousarray(bq.reshape(4, 128, 1))
    wv_r = np.ascontiguousarray(wv.reshape(CB, 128, HD))
    # o_w rows for this head group, regrouped [g, 128, C] in head-pair order
    wo = o_w[hg * HD:(hg + 1) * HD, :]        # [512, C]
    wo_r = np.ascontiguousarray(wo.reshape(4, 128, C))

    xT = np.ascontiguousarray(hidden_state[b].T)  # [C, T]
    return {
        "xT": xT.astype(bf16),
        "wqk": wqk_r.astype(bf16),
        "bq": bq_r.astype(np.float32),
        "wv": wv_r.astype(bf16),
        "wo": wo_r.astype(bf16),
    }


def _ensure_profile_hook():
    """Register the NTFF profiling hook that this container's antenv lacks.

    The axon boot code registers it via ``antenv.axon_hooks`` when that
    module exists; here we synthesize the module and point it at the same
    ctypes shim over libaxon_pjrt.so.
    """
    import types
    try:
        from antenv.axon_hooks import get_axon_ntff_profile_hook  # noqa: F401
        return
    except ImportError:
        pass
    try:
        import antenv
        from trn_agent_boot.trn_boot import _ntff_profile_via_ctypes
        hook = {"h": _ntff_profile_via_ctypes("/opt/axon/libaxon_pjrt.so")}
        mod = types.ModuleType("antenv.axon_hooks")
        mod.set_axon_ntff_profile_hook = lambda h: hook.__setitem__("h", h)
        mod.get_axon_ntff_profile_hook = lambda: hook["h"]
        sys.modules["antenv.axon_hooks"] = mod
        antenv.axon_hooks = mod
    except Exception as e:  # profiling is best-effort
        print(f"profile hook setup failed: {e}", flush=True)


def kernel(hidden_state, qkv_w, qkv_b, o_w, o_b):
    global _compiled
    hidden_state = np.asarray(hidden_state, dtype=np.float32)
    qkv_w = np.asarray(qkv_w, dtype=np.float32)
    qkv_b = np.asarray(qkv_b, dtype=np.float32)
    o_w = np.asarray(o_w, dtype=np.float32)
    o_b = np.asarray(o_b, dtype=np.float32)

    if _compiled is None:
        _compiled = _build()
    nc = _compiled

    in_maps = []
    for core in range(N_CORES):
        b, hg = core // 2, core % 2
        in_maps.append(_prep_core_inputs(hidden_state, qkv_w, qkv_b, o_w, b, hg))

    global LAST_EXEC_NS, LAST_TRACE
    kw = {}
    if TRACE:
        import tempfile
        _ensure_profile_hook()
        kw = dict(trace=True, tmpdir=tempfile.mkdtemp(prefix="bass_attn_trace_"))
    res = run_bass_kernel_spmd(nc, in_maps, core_ids=list(range(N_CORES)), **kw)
    LAST_EXEC_NS = res.exec_time_ns
    LAST_TRACE = res.instructions_and_trace

    # host-side gather: sum the two head-group partials per batch and add the
    # affine correction (v-bias pushed through Wo, plus o-bias).
    bv = qkv_b[2 * C:3 * C]                   # [C]
    corr = (bv @ o_w + o_b).astype(np.float32)
    out = np.empty((B, T, C), dtype=np.float32)
    for b in range(B):
        p0 = res.results[2 * b]["out_p"]
        p1 = res.results[2 * b + 1]["out_p"]
        out[b] = p0 + p1 + corr
    return out


# revision 32
# speedup vs baseline: 1.0119x; 1.0017x over previous
r"""Causal multi-head attention (B=4, T=2048, C=1024, H=16, D=64) on 8 TRN2 NeuronCores.

Sharding: core = (batch b, head-group hg).  b = core // 2, hg = core % 2.
Each core computes, for its batch, the attention-output contribution of its
8 heads, including the qkv projection restricted to those heads' columns and
the o-projection restricted to those heads' rows.  The two cores sharing a
batch produce partial sums of the o-projection; the host adds them together
with the (analytically folded) v-bias/o-bias correction.

Math notes:
  - k-bias contributes only q-row-constant score shifts, which cancel in
    softmax, so it is dropped; only the q bias is applied on device.
  - v bias and o bias are affine post-softmax:  (P@(V + 1 b_v^T))@Wo + b_o =
    (P@V)@Wo + (b_v@Wo + b_o), folded into a host-side correction row.
  - Softmax runs without max subtraction (scores are O(1)): e = exp(s/8).
    The denominator rides as PSUM rows 64-127 of the PV accumulation via 64
    all-ones columns appended to each V tile (lhsT M=128 costs nothing: the
    PE streams only the rhs free dim).

Engine schedule (the point of this version): the PE must never wait on the
softmax chain, and no side engine may become the bottleneck.
  - Lag-2 software pipeline in the attention inner loop: PV(tk-2) is issued
    after S(tk)/exp(tk), so every matmul's semaphores fire ~2 iterations
    early and consecutive matmuls overlap on the PE (fill/drain pipelining),
    like the projection phase achieves naturally.
  - PSUM: one shared pool of [128,1024] (2-bank) generations, bufs=3, used
    by S pairs, the qkv projection, and the o-projection; plus a
    single-buffered [128,1024] PV accumulator.  6 + 2 = 8 banks.
  - Both heads of a pair share one [128,1024] PSUM tile, so one exp
    instruction covers both S tiles (halves instruction overhead).
  - exp splits across two engines: ACT (table exp) takes most tiles; the
    DVE takes the triangular-masked diagonal blocks (a fused
    scalar_tensor_tensor computes a Schraudolph bit-trick exp:
    int16(s * 128*log2(e)/8 + bias) reinterpreted as bf16, where the bias
    tile holds 16249 on valid entries and -32000 on masked ones, which
    lands at -4e-36 ~ 0) plus overflow tiles when ACT would saturate.
  - Per head pair the PV PSUM finalize is deferred and staged into the
    NEXT pair's instruction stream: evacuations (ACT one head, DVE the
    other, freeing the single-buffered PSUM) and the GpSimd denominator
    moves go out after the next pair's first exps; the reciprocals +
    normalize multiplies two iterations later, when their inputs are long
    ready.  This keeps the DVE queue free of head-of-line blocking.
    GpSimd moves the denominators to partition base 0 because the
    custom-DVE reciprocal_approx_fast only works at base 0; the final
    normalize is a GpSimd multiply straight into bf16 O^T.
  - Chunk 0 (all-diagonal, tiny PE window) is emitted in the middle of the
    V projection with the remaining projection groups woven through its
    iterations as dependency-free PE filler; those tiles evacuate via ACT
    since chunk 0 loads the DVE.
  - o-projection of chunk c-1 interleaves one token-tile per head-pair
    finalize inside chunk c; its PSUM is evacuated by the ACT engine (DVE
    for the last four, when ACT is congested).
  - Startup: only bq + x^T gate the first matmul; x^T loads as two halves
    on the sync and ACT DMA rings in parallel, and Wv / Wo are issued
    behind the first two weight-block loads.
"""

import sys

sys.path.insert(0, "/opt/trn_rl_repo")

import numpy as np

import concourse.bass as bass
import concourse.tile as tile
from concourse import bacc, mybir
from concourse.bass_utils import run_bass_kernel_spmd
from concourse.masks import make_upper_triangular

B, T, C = 4, 2048, 1024
H = 16
D = C // H          # 64
HL = 8              # heads per core
HD = HL * D         # 512: local head dim
N_CORES = 8
CB = C // 128       # 8 c-tiles
TQ_CH = T // 512    # 4 query chunks
TK_TILES = T // 128  # 16 key tiles

F32 = mybir.dt.float32
I16 = mybir.dt.int16
BF16 = mybir.dt.bfloat16

# Schraudolph exp-to-bf16 constants: bits(e^x) ~ int16(x*184.66496 + 16249)
# (128/ln 2 mantissa scale, 127*128 exponent bias, -7.3 sawtooth centering).
EXP_SCALE = 184.66496 * 0.125     # includes the 1/sqrt(D) score scaling
EXP_OFF = 16249.0
MASK_OFF = -32000.0               # bitcasts to -4e-36: an effective zero

_compiled = None
TRACE = False          # set True (e.g. from test.py) to neuron-profile the run
LAST_EXEC_NS = None    # filled with max per-core exec_time_ns when TRACE
LAST_TRACE = None      # (insts, trace_path) when TRACE


def _build():
    nc = bacc.Bacc("TRN2", target_bir_lowering=False, debug=False,
                   num_devices=N_CORES)

    xT_ap = nc.dram_tensor("xT", [C, T], BF16, kind="ExternalInput").ap()
    # wqk[n] = 128-column block n of [Wq_shard | Wk_shard], laid out
    # [n, cb, ci, j]: contraction c = cb*128 + ci, output column j.
    wqk_ap = nc.dram_tensor("wqk", [8, CB, 128, 128], BF16, kind="ExternalInput").ap()
    bq_ap = nc.dram_tensor("bq", [4, 128, 1], F32, kind="ExternalInput").ap()
    wv_ap = nc.dram_tensor("wv", [CB, 128, HD], BF16, kind="ExternalInput").ap()
    # wo[g] = rows of Wo for head pair g (head 2g rows 0-63, head 2g+1 rows 64-127)
    wo_ap = nc.dram_tensor("wo", [4, 128, C], BF16, kind="ExternalInput").ap()
    out_ap = nc.dram_tensor("out_p", [T, C], F32, kind="ExternalOutput").ap()

    with tile.TileContext(nc) as tc:
        with (
            tc.tile_pool(name="const", bufs=1) as const_pool,
            tc.tile_pool(name="qkt", bufs=1) as qkt_pool,
            tc.tile_pool(name="v", bufs=1) as v_pool,
            tc.tile_pool(name="ot", bufs=1) as ot_pool,
            tc.tile_pool(name="wow", bufs=1) as wo_pool,
            tc.tile_pool(name="e", bufs=4) as e_pool,
            tc.tile_pool(name="rec", bufs=2) as rec_pool,
            tc.tile_pool(name="stg", bufs=6) as stg_pool,
            tc.tile_pool(name="ostg", bufs=3) as ostg_pool,
            tc.tile_pool(name="ps", bufs=3, space="PSUM") as ps_pool,
            tc.tile_pool(name="ps_o", bufs=1, space="PSUM") as ps_o_pool,
        ):
            bqb = const_pool.tile([128, 4, 1], F32)
            nc.sync.dma_start(bqb[:], bq_ap.rearrange("n p o -> p n o"))
            # x^T in four DMAs on four different engine DGE queues: the
            # sync-queue ring serializes its transfers, so spreading across
            # engines is what actually parallelizes the load
            XT = const_pool.tile([128, CB, T], BF16, name="xt")
            for lo, hi, eng in ((0, 4, nc.sync), (4, 8, nc.scalar)):
                eng.dma_start(
                    XT[:, lo:hi, :],
                    xT_ap[lo * 128:hi * 128, :].rearrange(
                        "(cb p) t -> p cb t", p=128),
                )

            # Diagonal-block bias tile for the fused DVE exp: EXP_OFF where
            # key_i <= query_j, MASK_OFF elsewhere.
            offs = const_pool.tile([128, 128], F32)
            make_upper_triangular(nc, offs, val=EXP_OFF - MASK_OFF, diag=True)
            nc.vector.tensor_scalar_add(offs[:], offs[:], MASK_OFF)

            QKT = [qkt_pool.tile([128, T], BF16, name=f"qkt{n}") for n in range(8)]
            # V layout [128, 8 heads, 128]: cols 0-63 = head values, cols
            # 64-127 = 1.0 so the PV matmul emits 64 copies of the softmax
            # denominator as PSUM rows 64-127 of the same group.
            V = [v_pool.tile([128, HL, 2 * D], BF16, name=f"v{t}")
                 for t in range(TK_TILES)]
            for t in range(TK_TILES):
                nc.gpsimd.memset(V[t][:, :, D:2 * D], 1.0)
            OT = [ot_pool.tile([128, T], BF16, name=f"ot{g}") for g in range(4)]
            WV = wv_pool = const_pool.tile([128, CB, HD], BF16, name="wv")
            WO = const_pool.tile([128, 4, C], BF16, name="wo")

            # ACT-engine budget per chunk (ns): ~88% of the chunk's PE
            # window minus the ACT-side staging/o-proj evacuations.
            act_cap = [12000.0, 23000.0, 38500.0, 52500.0]
            act_used = [0.0]

            def exp_issue(q0, dst, src, width, diag_block):
                """Emit exp(src*0.125) -> dst (bf16) on ACT or DVE."""
                if diag_block:
                    nc.vector.scalar_tensor_tensor(
                        dst.bitcast(I16), src, EXP_SCALE, offs[:],
                        mybir.AluOpType.mult, mybir.AluOpType.add,
                    )
                    return
                cost_act = width * 0.833 + 180
                if act_used[0] + cost_act <= act_cap[q0]:
                    act_used[0] += cost_act
                    nc.scalar.activation(
                        dst, src, mybir.ActivationFunctionType.Exp,
                        scale=0.125,
                    )
                else:
                    nc.vector.tensor_scalar(
                        dst.bitcast(I16), src, EXP_SCALE, EXP_OFF,
                        mybir.AluOpType.mult, mybir.AluOpType.add,
                    )

            pending_fin = []

            def flush_fin(stage=None):
                while pending_fin and (stage is None or pending_fin[0][1] <= stage):
                    pending_fin.pop(0)[0]()

            def attention_chunk(q0, boundary, iter_hook=None):
                """S/exp/PV + normalize for one 512-query chunk.

                boundary(hp) is called after each head pair's normalize
                chain is issued (used to interleave prior-chunk o-proj);
                iter_hook() after every inner iteration (used to weave
                independent PE work through chunk 0's short pipeline).
                """
                tq0 = q0 * 512
                ntk = q0 * 4 + 4
                act_used[0] = 0.0
                for hp in range(HL // 2):        # head pairs: their K=64 S
                    ha, hb = 2 * hp, 2 * hp + 1  # matmuls use disjoint PE
                    qt = QKT[hp]                 # row groups -> concurrent
                    kt = QKT[4 + hp]
                    # both heads' PV accumulate into one 2-bank tile:
                    # cols 0-511 head a, cols 512-1023 head b
                    pso = ps_o_pool.tile([128, 2 * 512], F32, name="pso")

                    def issue_pv(item):
                        ptk, pj0, pe = item
                        nc.tensor.matmul(
                            pso[:, pj0:512], V[ptk][:, ha, :], pe[:, pj0:512],
                            start=(ptk == 0), stop=(ptk == ntk - 1),
                        )
                        nc.tensor.matmul(
                            pso[:, 512 + pj0:1024], V[ptk][:, hb, :],
                            pe[:, 512 + pj0:1024],
                            start=(ptk == 0), stop=(ptk == ntk - 1),
                        )

                    pend = []
                    for tk in range(ntk):
                        r = tk - q0 * 4
                        j0 = r * 128 if r >= 0 else 0
                        pss = ps_pool.tile([128, 2 * 512], F32, name="ps")
                        nc.tensor.matmul(
                            pss[:, j0:512],
                            kt[0:64, tk * 128:(tk + 1) * 128],
                            qt[0:64, tq0 + j0:tq0 + 512],
                            start=True, stop=True,
                        )
                        nc.tensor.matmul(
                            pss[:, 512 + j0:1024],
                            kt[64:128, tk * 128:(tk + 1) * 128],
                            qt[64:128, tq0 + j0:tq0 + 512],
                            start=True, stop=True,
                        )
                        e_t = e_pool.tile([128, 2 * 512], BF16, name="e_t")
                        if r >= 0:
                            for h0 in (0, 512):
                                exp_issue(q0, e_t[:, h0 + j0:h0 + j0 + 128],
                                          pss[:, h0 + j0:h0 + j0 + 128],
                                          128, True)
                                if j0 + 128 < 512:
                                    exp_issue(q0, e_t[:, h0 + j0 + 128:h0 + 512],
                                              pss[:, h0 + j0 + 128:h0 + 512],
                                              512 - j0 - 128, False)
                        else:
                            # one instruction covers both heads
                            exp_issue(q0, e_t[:], pss[:], 1024, False)
                        pend.append((tk, j0, e_t))
                        if len(pend) > 2:
                            issue_pv(pend.pop(0))
                        if tk == 1:
                            # the previous head pair's finalize is staged
                            # into this pair's stream: evacuations (freeing
                            # the PV PSUM) go out now, after this pair's
                            # first exps are in the DVE queue...
                            flush_fin(1)
                        elif tk == 3:
                            # ...and the reciprocals two iterations later,
                            # when their denominator inputs are long ready,
                            # so they retire instantly instead of
                            # head-of-line-blocking queued DVE exps
                            flush_fin(2)
                        if iter_hook is not None:
                            iter_hook()
                    for item in pend:
                        issue_pv(item)
                    tail = q0 == TQ_CH - 1
                    stgs = []

                    def fin1(pso=pso, tail=tail):
                        # evacuate raw PV+denominator to SBUF staging (frees
                        # the single-buffered PSUM; ACT head a, DVE head b),
                        # and start the GpSimd denominator moves down to
                        # partition base 0 (the custom-DVE
                        # reciprocal_approx_fast only works at base 0)
                        stg_a = stg_pool.tile([128, 512], F32, name="stg_a")
                        stg_b = stg_pool.tile([128, 512], F32, name="stg_b")
                        nc.scalar.copy(stg_a[:], pso[:, 0:512])
                        nc.vector.tensor_copy(stg_b[:], pso[:, 512:1024])
                        for po, stg in ((0, stg_a), (64, stg_b)):
                            den0 = rec_pool.tile([64, 512], F32, name=f"den{po}")
                            (nc.vector if tail else nc.gpsimd).tensor_copy(
                                den0[:], stg[64:128, :])
                            stgs.append((po, stg, den0))

                    def fin2(hp=hp, tail=tail):
                        for po, stg, den0 in stgs:
                            rec = rec_pool.tile([64, 512], F32, name=f"rec{po}")
                            nc.vector.reciprocal_approx_fast(rec[:], den0[:])
                            (nc.vector if tail else nc.gpsimd).tensor_mul(
                                OT[hp][po:po + 64, tq0:tq0 + 512],
                                stg[0:64, :], rec[:],
                            )
                        boundary(hp)
                    if q0 == TQ_CH - 1 and hp == HL // 2 - 1:
                        fin1()
                        fin2()
                    else:
                        pending_fin.append((fin1, 1))
                        pending_fin.append((fin2, 2))

            def oproj(tt, evac=None):
                psp = ps_pool.tile([128, 2 * 512], F32, name="ps")
                for half in range(2):
                    n0 = half * 512
                    for g in range(4):
                        nc.tensor.matmul(
                            psp[:, n0:n0 + 512],
                            OT[g][:, tt * 128:(tt + 1) * 128],
                            WO[:, g, n0:n0 + 512],
                            start=(g == 0), stop=(g == 3),
                        )
                ob = ostg_pool.tile([128, 2 * 512], F32, name="ob")
                if evac is None:
                    nc.scalar.copy(ob[:], psp[:])
                    act_used[0] += 1000
                else:
                    evac(ob[:], psp[:])
                nc.sync.dma_start(out_ap[tt * 128:(tt + 1) * 128, :], ob[:])

            # ---------------- projection phase (+ chunk 0) ----------------
            with (
                tc.tile_pool(name="wqk", bufs=2) as wqk_pool,
            ):
                # QK^T = W^T @ x^T, output rows = qk columns (col block n)
                for n in range(8):
                    wt = wqk_pool.tile([128, CB, 128], BF16)
                    nc.sync.dma_start(wt[:], wqk_ap[n].rearrange("cb ci j -> ci cb j"))
                    if n == 1:
                        # queue the big weight loads behind the first blocks
                        nc.sync.dma_start(
                            WV[:], wv_ap.rearrange("cb p h -> p cb h"))
                    elif n == 2:
                        nc.sync.dma_start(
                            WO[:], wo_ap.rearrange("g p c -> p g c"))
                    for q0 in range(TQ_CH):
                        ps = ps_pool.tile([128, 2 * 512], F32, name="ps")
                        for cb in range(CB):
                            nc.tensor.matmul(
                                ps[:, 0:512], wt[:, cb, :],
                                XT[:, cb, q0 * 512:(q0 + 1) * 512],
                                start=(cb == 0), stop=(cb == CB - 1),
                            )
                        dst = QKT[n][:, q0 * 512:(q0 + 1) * 512]
                        if n < 4:
                            nc.vector.tensor_scalar_add(
                                dst, ps[:, 0:512], bqb[:, n, :])
                        else:
                            # plain cast: use the otherwise-idle ACT engine
                            nc.scalar.copy(dst, ps[:, 0:512])

                # V = x @ Wv (natural layout)
                def vproj(tt):
                    ps = ps_pool.tile([128, 2 * 512], F32, name="ps")
                    for cb in range(CB):
                        nc.tensor.matmul(
                            ps[:, 0:512], XT[:, cb, tt * 128:(tt + 1) * 128],
                            WV[:, cb, :],
                            start=(cb == 0), stop=(cb == CB - 1),
                        )
                    # tiles woven through chunk 0 evacuate via ACT: the DVE
                    # is loaded with chunk 0's fused mask-exps there
                    (nc.scalar.copy if tt >= 4 else nc.vector.tensor_copy)(
                        V[tt][:, :, 0:D],
                        ps[:, 0:512].rearrange("p (h d) -> p h d", h=HL),
                    )

                for tt in range(4):
                    vproj(tt)
                # chunk 0 runs with the remaining V projection woven through
                # its iterations: the dependency-free projection groups keep
                # the PE streaming across chunk 0's short softmax pipeline
                pending_v = list(range(4, TK_TILES))
                it_count = [0]

                def weave():
                    it_count[0] += 1
                    if pending_v and it_count[0] % 4 != 0:
                        vproj(pending_v.pop(0))

                attention_chunk(0, lambda hp: None, weave)
                for tt in pending_v:
                    vproj(tt)

            # ---------------- attention + output projection ----------------
            for q0 in range(1, TQ_CH):
                attention_chunk(q0, lambda hp, c=q0: oproj(4 * (c - 1) + hp))
            flush_fin()
            for tt in range(12, 16):
                oproj(tt, evac=nc.vector.tensor_copy)

    nc.compile()
    return nc


def _prep_core_inputs(hidden_state, qkv_w, qkv_b, o_w, b, hg):
    """Build the per-core input map for batch b, head group hg."""
    import ml_dtypes
    bf16 = ml_dtypes.bfloat16
    s = slice(hg * HD, (hg + 1) * HD)
    wq = qkv_w[:, 0 * C:1 * C][:, s]          # [C, 512]
    wk = qkv_w[:, 1 * C:2 * C][:, s]          # [C, 512]
    wv = qkv_w[:, 2 * C:3 * C][:, s]          # [C, 512]
    bq = qkv_b[0 * C:1 * C][s]                # [512]

    wqk = np.concatenate([wq, wk], axis=1)    # [C, 1024]
    # [n, cb, ci, j]
    wqk_r = np.ascontiguousarray(
        wqk.reshape(CB, 128, 8, 128).transpose(2, 0, 1, 3)
    )
    bq_r = np.ascontiguousarray(bq.reshape(4, 128, 1))
    wv_r = np.ascontiguousarray(wv.reshape(CB, 128, HD))
    # o_w rows for this head group, regrouped [g, 128, C] in head-pair order
    wo = o_w[hg * HD:(hg + 1) * HD, :]        # [512, C]
    wo_r = np.ascontiguousarray(wo.reshape(4, 128, C))

    xT = np.ascontiguousarray(hidden_state[b].T)  # [C, T]
    return {
        "xT": xT.astype(bf16),
        "wqk": wqk_r.astype(bf16),
        "bq": bq_r.astype(np.float32),
        "wv": wv_r.astype(bf16),
        "wo": wo_r.astype(bf16),
    }


def _ensure_profile_hook():
    """Register the NTFF profiling hook that this container's antenv lacks.

    The axon boot code registers it via ``antenv.axon_hooks`` when that
    module exists; here we synthesize the module and point it at the same
    ctypes shim over libaxon_pjrt.so.
    """
    import types
    try:
        from antenv.axon_hooks import get_axon_ntff_profile_hook  # noqa: F401
        return
    except ImportError:
        pass
    try:
        import antenv
        from trn_agent_boot.trn_boot import _ntff_profile_via_ctypes
        hook = {"h": _ntff_profile_via_ctypes("/opt/axon/libaxon_pjrt.so")}
        mod = types.ModuleType("antenv.axon_hooks")
        mod.set_axon_ntff_profile_hook = lambda h: hook.__setitem__("h", h)
        mod.get_axon_ntff_profile_hook = lambda: hook["h"]
        sys.modules["antenv.axon_hooks"] = mod
        antenv.axon_hooks = mod
    except Exception as e:  # profiling is best-effort
        print(f"profile hook setup failed: {e}", flush=True)


def kernel(hidden_state, qkv_w, qkv_b, o_w, o_b):
    global _compiled
    hidden_state = np.asarray(hidden_state, dtype=np.float32)
    qkv_w = np.asarray(qkv_w, dtype=np.float32)
    qkv_b = np.asarray(qkv_b, dtype=np.float32)
    o_w = np.asarray(o_w, dtype=np.float32)
    o_b = np.asarray(o_b, dtype=np.float32)

    if _compiled is None:
        _compiled = _build()
    nc = _compiled

    in_maps = []
    for core in range(N_CORES):
        b, hg = core // 2, core % 2
        in_maps.append(_prep_core_inputs(hidden_state, qkv_w, qkv_b, o_w, b, hg))

    global LAST_EXEC_NS, LAST_TRACE
    kw = {}
    if TRACE:
        import tempfile
        _ensure_profile_hook()
        kw = dict(trace=True, tmpdir=tempfile.mkdtemp(prefix="bass_attn_trace_"))
    res = run_bass_kernel_spmd(nc, in_maps, core_ids=list(range(N_CORES)), **kw)
    LAST_EXEC_NS = res.exec_time_ns
    LAST_TRACE = res.instructions_and_trace

    # host-side gather: sum the two head-group partials per batch and add the
    # affine correction (v-bias pushed through Wo, plus o-bias).
    bv = qkv_b[2 * C:3 * C]                   # [C]
    corr = (bv @ o_w + o_b).astype(np.float32)
    out = np.empty((B, T, C), dtype=np.float32)
    for b in range(B):
        p0 = res.results[2 * b]["out_p"]
        p1 = res.results[2 * b + 1]["out_p"]
        out[b] = p0 + p1 + corr
    return out
